# revision 10
# baseline (speedup 1.0000x reference)
"""AFT-full attention kernel for Trainium2, 8 NeuronCores, data-parallel over batch.

Problem (per reference):
    q = x @ Wq.T + bq ; k = x @ Wk.T + bk ; v = x @ Wv.T + bv
    ek = exp(k); eb = exp(pos_bias)
    num = einsum('ij,bjd->bid', eb, ek*v); den = einsum('ij,bjd->bid', eb, ek)
    out = sigmoid(q) * num / den

Shapes: x [32, 1024, 512], W* [512, 512], pos_bias [1024, 1024].

Strategy: batch-data-parallel, 4 batches per core, no collectives.
bf16 tensor-engine compute. The host passes x / W / pos_bias already
transposed (numpy), so every tensor lands in SBUF in the orientation the
TensorEngine needs (contraction dim on partitions) with plain DMAs --
no DMA-transpose (256B packet floods) and no on-chip transposes.

ScalarE function usage is phase-batched (a run of Exp ops, then a run of
Sigmoid ops per batch) because every activation-function switch reloads
the ScalarE LUT (~1.3us).

Host-side dispatch: when pos_bias is a constant matrix (as in the AFT
init, pos_bias = ones), exp(pos_bias) is rank-1 and the (n,n)x(n,d)
contraction reduces EXACTLY to column sums (the exp(c) factor cancels
between num and den); a much smaller graph handles that case. The
general graph handles arbitrary pos_bias.
"""

import sys

sys.path.insert(0, "/opt/trn_rl_repo")

import numpy as np

P = 128
D = 512  # d_model
N = 1024  # sequence length
BS = 32
CORES = 8
BPC = BS // CORES  # batches per core
NT = N // P  # 8 n-tiles per batch
ROWS = BPC * N  # 4096 rows of x per core

_CACHE = {}


def _build(kin, rank1):
    import concourse.tile as tile
    from concourse import bacc, mybir
    from concourse.masks import make_identity
    from contextlib import ExitStack

    f32 = mybir.dt.float32
    bf16 = mybir.dt.bfloat16
    AF = mybir.ActivationFunctionType
    ALU = mybir.AluOpType

    dkt = kin // P  # k-tiles for projections

    nc = bacc.Bacc("TRN2", target_bir_lowering=False, debug=False, num_devices=CORES)

    xT_ext = nc.dram_tensor("xT", [kin, ROWS], f32, kind="ExternalInput")
    wT_ext = [
        nc.dram_tensor(nm, [kin, D], f32, kind="ExternalInput")
        for nm in ("WqT", "WkT", "WvT")
    ]
    pbT_ext = None
    if not rank1:
        pbT_ext = nc.dram_tensor("pbT", [N, N], f32, kind="ExternalInput")
    out_ext = nc.dram_tensor("out", [ROWS, D], f32, kind="ExternalOutput")

    with tile.TileContext(nc) as tc, ExitStack() as ctx:
        prep = ctx.enter_context(tc.tile_pool(name="prep", bufs=4))
        res = ctx.enter_context(tc.tile_pool(name="res", bufs=1))
        xtp = ctx.enter_context(tc.tile_pool(name="xtp", bufs=2))
        ekp = ctx.enter_context(tc.tile_pool(name="ekp", bufs=2))
        sqp = ctx.enter_context(tc.tile_pool(name="sqp", bufs=2))
        tmp = ctx.enter_context(tc.tile_pool(name="tmp", bufs=3))
        outp = ctx.enter_context(tc.tile_pool(name="outp", bufs=3))
        psum = ctx.enter_context(tc.tile_pool(name="psum", bufs=2, space="PSUM"))

        ident = res.tile([P, P], bf16, name="ident")
        make_identity(nc, ident[:])
        # dummy transposes: keep the PE busy during the DMA lead-in so the
        # HAM clock gate opens (1.2 -> 2.4 GHz) before real matmuls start
        ps_warm = psum.tile([P, P], bf16, tag="ps_tr", name="ps_warm")
        for _ in range(48):
            nc.tensor.transpose(ps_warm[:], ident[:], ident[:])
        if rank1:
            ones_col = res.tile([P, 1], bf16, name="ones_col")
            nc.gpsimd.memset(ones_col[:], 1.0)
            ones_row = res.tile([1, P], f32, name="ones_row")
            nc.gpsimd.memset(ones_row[:], 1.0)

        # ---- W: load pre-transposed [din, dout] f32, cast to bf16 ----
        wt = []
        for wi in range(3):
            per_w = []
            for dt in range(dkt):
                w_t = prep.tile([P, D], f32, tag="w_ld", name=f"wld{wi}_{dt}")
                nc.sync.dma_start(w_t[:], wT_ext[wi][dt * P : (dt + 1) * P, :])
                t = res.tile([P, D], bf16, name=f"wt{wi}_{dt}")
                nc.vector.tensor_copy(t[:], w_t[:])
                per_w.append(t)
            wt.append(per_w)

        # ---- eb (general path): EBT[j] = exp(pbT[j-tile]) [j on partitions]
        ebt = []
        if not rank1:
            for j in range(NT):
                pb_t = prep.tile([P, N], f32, tag="pb_ld", name=f"pbld{j}")
                nc.scalar.dma_start(pb_t[:], pbT_ext[j * P : (j + 1) * P, :])
                t = res.tile([P, N], bf16, name=f"ebt{j}")
                nc.scalar.activation(t[:], pb_t[:], AF.Exp)
                ebt.append(t)

        def make_xt(b):
            """load xT[:, batch b] f32 tiles, cast bf16 -> xt[dt] [128(d),1024(n)]"""
            xt = []
            for dt in range(dkt):
                x_t = prep.tile([P, N], f32, tag="x_ld", name=f"xld{b}_{dt}")
                nc.scalar.dma_start(
                    x_t[:], xT_ext[dt * P : (dt + 1) * P, b * N : (b + 1) * N]
                )
                t = xtp.tile([P, N], bf16, tag=f"xt{dt}", name=f"xt{b}_{dt}")
                nc.vector.tensor_copy(t[:], x_t[:])
                xt.append(t)
            return xt

        xt = make_xt(0)

        for b in range(BPC):
            r0 = b * N
            ek = [None] * NT
            ekv = [None] * NT
            q_sb = [None] * NT
            exp_inst = None
            # projections; ACT does only Exp in this phase
            for ni in range(NT):
                q_ps = psum.tile([P, D], f32, tag="ps_a", name=f"qps{b}_{ni}")
                k_ps = psum.tile([P, D], f32, tag="ps_b", name=f"kps{b}_{ni}")
                v_ps = psum.tile([P, D], f32, tag="ps_c", name=f"vps{b}_{ni}")
                nsl = slice(ni * P, (ni + 1) * P)
                for dt in range(dkt):
                    st, sp = dt == 0, dt == dkt - 1
                    nc.tensor.matmul(q_ps[:], xt[dt][:, nsl], wt[0][dt][:], start=st, stop=sp)
                    nc.tensor.matmul(k_ps[:], xt[dt][:, nsl], wt[1][dt][:], start=st, stop=sp)
                    nc.tensor.matmul(v_ps[:], xt[dt][:, nsl], wt[2][dt][:], start=st, stop=sp)
                q_sb[ni] = sqp.tile([P, D], bf16, tag=f"qsb{ni}", name=f"qsb{b}_{ni}")
                nc.vector.tensor_copy(q_sb[ni][:], q_ps[:])
                ek[ni] = ekp.tile([P, D], bf16, tag=f"ek{ni}", name=f"ek{b}_{ni}")
                exp_inst = nc.scalar.activation(ek[ni][:], k_ps[:], AF.Exp)
                ekv[ni] = ekp.tile([P, D], bf16, tag=f"ekv{ni}", name=f"ekv{b}_{ni}")
                nc.vector.tensor_mul(ekv[ni][:], ek[ni][:], v_ps[:])

            # batched sigmoid phase (one LUT switch per batch); pin the
            # sigmoids after the batch's last Exp so the LUT only swaps twice
            sq = [None] * NT
            for ni in range(NT):
                sq[ni] = sqp.tile([P, D], bf16, tag=f"sq{ni}", name=f"sq{b}_{ni}")
                sig = nc.scalar.activation(sq[ni][:], q_sb[ni][:], AF.Sigmoid)
                tile.add_dep_helper(
                    sig.ins, exp_inst.ins, sync=False, reason="batch sigmoids"
                )

            if rank1:
                # column sums over j: num_row = 1^T @ ekv ; den_row = 1^T @ ek
                ns_ps = psum.tile([1, D], f32, tag="ps_c", name=f"nsps{b}")
                ds_ps = psum.tile([1, D], f32, tag="ps_c", name=f"dsps{b}")
                for j in range(NT):
                    st, sp = j == 0, j == NT - 1
                    nc.tensor.matmul(ns_ps[:], ones_col[:], ekv[j][:], start=st, stop=sp)
                    nc.tensor.matmul(ds_ps[:], ones_col[:], ek[j][:], start=st, stop=sp)
                nr = tmp.tile([1, D], f32, tag="nr", name=f"nr{b}")
                nc.vector.tensor_copy(nr[:], ns_ps[:])
                dr_inv = tmp.tile([1, D], f32, tag="dr", name=f"dr{b}")
                nc.vector.reciprocal_approx_fast(dr_inv[:], ds_ps[:])
                r_row = tmp.tile([1, D], f32, tag="rr", name=f"rr{b}")
                nc.vector.tensor_mul(r_row[:], nr[:], dr_inv[:])
                # broadcast r_row over 128 partitions with a K=1 matmul
                bc_ps = psum.tile([P, D], f32, tag="ps_c", name=f"bcps{b}")
                nc.tensor.matmul(bc_ps[:], ones_row[:], r_row[:], start=True, stop=True)
                r_b = tmp.tile([P, D], f32, tag="rb", bufs=2, name=f"rb{b}")
                nc.vector.tensor_copy(r_b[:], bc_ps[:])

            if b + 1 < BPC:
                xt = make_xt(b + 1)  # overlaps the epilogue below

            if rank1:
                # out[i-tile] = sq[i] * r_b
                for ii in range(NT):
                    o_t = outp.tile([P, D], f32, tag="ot", name=f"ot{b}_{ii}")
                    nc.vector.tensor_mul(o_t[:], sq[ii][:], r_b[:])
                    nc.sync.dma_start(
                        out_ext[r0 + ii * P : r0 + (ii + 1) * P, :], o_t[:]
                    )
            else:
                # AFT contraction: num/den per i-tile over j-tiles
                for ii in range(NT):
                    num_ps = psum.tile([P, D], f32, tag="ps_a", name=f"nps{b}_{ii}")
                    den_ps = psum.tile([P, D], f32, tag="ps_b", name=f"dps{b}_{ii}")
                    isl = slice(ii * P, (ii + 1) * P)
                    for j in range(NT):
                        st, sp = j == 0, j == NT - 1
                        nc.tensor.matmul(num_ps[:], ebt[j][:, isl], ekv[j][:], start=st, stop=sp)
                        nc.tensor.matmul(den_ps[:], ebt[j][:, isl], ek[j][:], start=st, stop=sp)
                    rec = tmp.tile([P, D], f32, tag="rec", name=f"rec{b}_{ii}")
                    nc.vector.reciprocal_approx_fast(rec[:], den_ps[:])
                    t1 = tmp.tile([P, D], f32, tag="t1", name=f"t1_{b}_{ii}")
                    nc.vector.scalar_tensor_tensor(
                        t1[:], num_ps[:], 1.0, rec[:], ALU.mult, ALU.mult
                    )
                    o_t = outp.tile([P, D], f32, tag="ot", name=f"ot{b}_{ii}")
                    nc.vector.tensor_mul(o_t[:], t1[:], sq[ii][:])
                    nc.sync.dma_start(
                        out_ext[r0 + ii * P : r0 + (ii + 1) * P, :], o_t[:]
                    )

    nc.compile()
    return nc


def _get_nc(kin, rank1):
    key = (kin, rank1)
    if key not in _CACHE:
        _CACHE[key] = _build(kin, rank1)
    return _CACHE[key]


def kernel(x, Wq, bq, Wk, bk, Wv, bv, pos_bias):
    from concourse.bass_utils import run_bass_kernel_spmd

    x = np.asarray(x, dtype=np.float32)
    pos_bias = np.asarray(pos_bias, dtype=np.float32)
    no_bias = not (np.any(bq) or np.any(bk) or np.any(bv))
    # exp(c*ones) is rank-1 and cancels between num and den -> column sums
    rank1 = bool(pos_bias.size) and bool(np.all(pos_bias == pos_bias.flat[0]))

    if no_bias:
        kin = D
        xk = x.reshape(BS * N, D)
        wqT = np.ascontiguousarray(np.asarray(Wq, np.float32).T)
        wkT = np.ascontiguousarray(np.asarray(Wk, np.float32).T)
        wvT = np.ascontiguousarray(np.asarray(Wv, np.float32).T)
    else:
        # fold biases in by augmenting the contraction dim
        kin = D + P
        xk = np.zeros((BS * N, kin), np.float32)
        xk[:, :D] = x.reshape(BS * N, D)
        xk[:, D] = 1.0

        def augT(W, bvec):
            Wa = np.zeros((kin, D), np.float32)
            Wa[:D, :] = np.asarray(W, np.float32).T
            Wa[D, :] = bvec
            return Wa

        wqT, wkT, wvT = augT(Wq, bq), augT(Wk, bk), augT(Wv, bv)

    pbT = None if rank1 else np.ascontiguousarray(pos_bias.T)

    nc = _get_nc(kin, rank1)
    in_maps = []
    for c in range(CORES):
        m = {
            "xT": np.ascontiguousarray(xk[c * ROWS : (c + 1) * ROWS].T),
            "WqT": wqT,
            "WkT": wkT,
            "WvT": wvT,
        }
        if not rank1:
            m["pbT"] = pbT
        in_maps.append(m)
    res = run_bass_kernel_spmd(nc, in_maps, core_ids=list(range(CORES)))
    out = np.concatenate([res.results[c]["out"] for c in range(CORES)], axis=0)
    return out.reshape(BS, N, D)
